# revision 2
# baseline (speedup 1.0000x reference)
"""Trainium2 Bass kernel for nn_ConstraintLoss (mse + dynamics/obstacle loss).

Data-parallel over 8 cores (131072 rows -> 16384/core). Per core the shard
is processed in 5 row-tiles (K rows/partition: 32,32,32,16,16); partition p
owns shard rows [128p, 128p+128) so every tile load is one contiguous
K*960B block per partition.

Math per row (telescoped; see reference):
  x = p[0:160] as (40,4)[px,py,th,v]; u = p[160:240] as (40,2)[a,w]
  resid/DT = (x39 - x0)/DT - q,  q = [q_c, q_s, q_w, q_a]
    q_c = v0 cos th0 + sum_{j<39} v_j cos th_j   (q_s with sin)
    q_w = sum_j w_j, q_a = sum_j a_j             (j < 40)
  dyn_err = DT * ||resid/DT||
  obst_err = sum_{k,j} sqrt((px_j-ox_k)^2+(py_j-oy_k)^2) - 40*sum_k (r_k+2)^2
  out = mean(diff^2) + mean(dyn_err + obst_err)

Engine split (HW-measured per-op costs):
  - loads: SWDGE fp32->bf16 cast DMA, all descriptors emitted early so the
    HBM stream never gaps (deep-queued DMAs measured at ~400 GB/s read)
  - DVE: diff (2x, in-place into p tile), half of dx, custom SQSQADD
    (d2 = dx^2+dy^2 in one 1x op), sin/cos range wraps via the custom
    ADD_RANGE_WRAP op, vcs mul (2x), the j-reductions (1x)
  - GpSimd: dy + other half of dx (broadcast subs), BASE, DMA emissions
  - ACT: mse Square+accum, Sin, v-broadcast, X39 snapshot, (r+2)^2, sqrt
    batches; sin work all precedes sqrt work -> exactly 1 table switch
Trig: sin(th)=Sin(wrap(th)), cos(th)=Sin(wrap(th+pi/2)), wrap = one
custom-DVE op mapping into [-pi,pi] (valid for |arg|<3pi; |th|<6).
"""

from contextlib import ExitStack

import numpy as np

import concourse.bacc as bacc
import concourse.bass as bass
import concourse.tile as tile
import concourse.dve_ops as dve_ops
from concourse.dve_spec import Spec, Src0, Src1, sq
from concourse import mybir
from concourse.bass_utils import run_bass_kernel_spmd

N_CORES = 8
B = 131072
BC = B // N_CORES            # 16384 rows per core
P = 128                      # SBUF partitions
RPP = BC // P                # 128 rows per partition
KS = [32, 32, 32, 16, 16]    # rows-per-partition per tile
OFFS = [0, 32, 64, 96, 112]
NT = len(KS)
DT = 0.25
CAR_WIDTH = 2.0
N_OBST = 3
NJ = 40
PI = float(np.pi)
TWO_PI = float(2.0 * np.pi)
HALF_PI = float(np.pi / 2.0)
F32 = mybir.dt.float32
BF16 = mybir.dt.bfloat16
KMAX = 32

# out columns: DY (dyn_err/DT per group) 128, mse sums 5, obstacle dist
# sums 2, (r+2)^2 sums 5
OUT_COLS = RPP + NT + 2 + NT  # 140


def _bcast(ap, dim_idx, count):
    """Insert a step-0 (broadcast) dim at position dim_idx of ap's dim list."""
    dims = [list(d) for d in ap.ap]
    dims.insert(dim_idx, [0, count])
    return bass.AP(tensor=ap.tensor, offset=ap.offset, ap=dims)


def _register_custom(name, body, reference, shas):
    """Register a custom DveOp (idempotent); discover shas if pinned ones drift."""
    for op in dve_ops.OPS:
        if op.name == name:
            return op
    spec = Spec(body=body, reference=reference)
    row = dve_ops._CUSTOM_DVE_ROW_BASE + len(dve_ops.OPS)
    dve_ops._SUB_OPCODE_FOR_NAME[name] = row
    op = dve_ops.DveOp(name, spec, False, dict(shas))
    # validate pinned shas; rediscover on drift (concourse version skew)
    import re
    for ver in ("v3", "v4"):
        try:
            op.compile(ver)
        except ValueError as e:
            m = re.search(r"v\d: (\w{16})", str(e))
            shas = dict(shas)
            shas[ver] = m.group(1)
            dve_ops._COMPILE_CACHE.pop((name, ver), None)
            op = dve_ops.DveOp(name, spec, False, shas)
    dve_ops.OPS.append(op)
    dve_ops.CUSTOM_DVE_SPECS[name] = spec
    return op


def _sqsqadd():
    return _register_custom(
        "SQSQADD_ANT", sq(Src0) + sq(Src1),
        lambda in0, in1, s0, s1, imm2: (
            in0.astype(np.float32) ** 2 + in1.astype(np.float32) ** 2),
        {"v3": "cd4bd6e1c27efd14", "v4": "121e32d8332f5047"})


def build_nc():
    sqsqadd = _sqsqadd()
    nc = bacc.Bacc()
    pred = nc.declare_dram_parameter("predictions", [BC, 240], F32, isOutput=False)
    tgt = nc.declare_dram_parameter("targets", [BC, 240], F32, isOutput=False)
    inp = nc.declare_dram_parameter("inputs", [BC, 13], F32, isOutput=False)
    out = nc.declare_dram_parameter("out", [P, OUT_COLS], F32, isOutput=True)

    # partition p owns shard rows [RPP*p, RPP*(p+1))
    predv = pred[:].rearrange("(p r) c -> p r c", p=P, r=RPP)
    tgtv = tgt[:].rearrange("(p r) c -> p r c", p=P, r=RPP)
    inpv = inp[:].rearrange("(p r) c -> p r c", p=P, r=RPP)

    with tile.TileContext(nc) as tc, ExitStack() as ctx:
        pp = ctx.enter_context(tc.tile_pool(name="pp", bufs=3))
        tp = ctx.enter_context(tc.tile_pool(name="tp", bufs=2))
        ip = ctx.enter_context(tc.tile_pool(name="ip", bufs=3))
        sc = ctx.enter_context(tc.tile_pool(name="sc", bufs=2))
        per = ctx.enter_context(tc.tile_pool(name="per", bufs=1))

        CPOS = per.tile([P, 1], F32)          # +pi/2 (bias for initial table load)
        CW = per.tile([P, 1], F32)            # +CAR_WIDTH
        nc.vector.memset(CPOS[:], HALF_PI)
        nc.vector.memset(CW[:], CAR_WIDTH)
        TRASH1 = per.tile([P, 1], F32)
        # force the trig table load now (during the first DMA)
        nc.scalar.activation(out=TRASH1[:], in_=CPOS[:],
                             func=mybir.ActivationFunctionType.Sin)

        Q = per.tile([P, RPP, 4], F32)        # [q_c, q_s, q_w, q_a] per group
        X39 = per.tile([P, RPP, 4], F32)      # last state per group
        D2 = per.tile([P, RPP * N_OBST * NJ], BF16)  # staged dist^2
        DY2 = per.tile([P, RPP], F32)         # ||resid/DT||^2 per group
        OUT = per.tile([P, OUT_COLS], F32)
        RTRS = per.tile([P, KMAX, N_OBST], BF16)  # (r+2)^2 trash output

        # per-tile intermediates (pool-cycled, sliced to K)
        def tiles(t):
            K = KS[t]
            p_t = pp.tile([P, KMAX, 240], BF16, name=f"p_{t}", tag="p")
            t_t = tp.tile([P, KMAX, 240], BF16, name=f"t_{t}", tag="t")
            i_t = ip.tile([P, KMAX, 13], BF16, name=f"i_{t}", tag="i")
            return K, p_t, t_t, i_t

        tl = [tiles(t) for t in range(NT)]

        def emit_loads(t):
            K, p_t, t_t, i_t = tl[t]
            o = OFFS[t]
            nc.gpsimd.dma_start(out=p_t[:, 0:K], in_=predv[:, o:o + K])
            nc.gpsimd.dma_start(out=i_t[:, 0:K], in_=inpv[:, o:o + K])
            nc.gpsimd.dma_start(out=t_t[:, 0:K], in_=tgtv[:, o:o + K])

        # tiles 0-2: p/i fresh buffers; t0,t1 fresh (tp bufs=2) - t2 deferred
        emit_loads(0)
        emit_loads(1)
        K2, p2_t, t2_t, i2_t = tl[2]
        nc.gpsimd.dma_start(out=p2_t[:, 0:K2], in_=predv[:, OFFS[2]:OFFS[2] + K2])
        nc.gpsimd.dma_start(out=i2_t[:, 0:K2], in_=inpv[:, OFFS[2]:OFFS[2] + K2])

        basel = [None] * NT   # per-tile BASE tiles (gpsimd output)
        resl = [None] * NT

        def phase_b_finish(t):
            """RES/RES2/DY2 for tile t (after BASE_t exists)."""
            K = KS[t]
            o = OFFS[t]
            ts = slice(o, o + K)
            RES = sc.tile([P, KMAX, 4], F32, name=f"res_{t}", tag="res")
            nc.vector.scalar_tensor_tensor(
                out=RES[:, 0:K], in0=basel[t][:, 0:K], scalar=1.0 / DT,
                in1=Q[:, ts, :], op0=mybir.AluOpType.mult,
                op1=mybir.AluOpType.subtract)
            resl[t] = RES
            nc.scalar.activation(out=RES[:, 0:K], in_=RES[:, 0:K],
                                 func=mybir.ActivationFunctionType.Square)
            nc.vector.reduce_sum(out=DY2[:, ts], in_=RES[:, 0:K],
                                 axis=mybir.AxisListType.X)

        for t in range(NT):
            K, p_t, t_t, i_t = tl[t]
            o = OFFS[t]
            ts = slice(o, o + K)
            KH = K // 2

            xv = p_t[:, 0:K, 0:160].rearrange("p g (j f) -> p g j f", f=4)
            uv = p_t[:, 0:K, 160:240].rearrange("p g (j f) -> p g j f", f=2)
            ov = i_t[:, 0:K, 4:13].rearrange("p g (k f) -> p g k f", f=3)
            th38 = xv[:, :, 0:39, 2]
            v38 = xv[:, :, 0:39, 3]

            ws = sc.tile([P, KMAX, 2, 40], F32, name=f"ws_{t}", tag="ws")
            cs = sc.tile([P, KMAX, 2, 40], BF16, name=f"cs_{t}", tag="cs")
            vd = sc.tile([P, KMAX, 2, 40], BF16, name=f"vd_{t}", tag="vd")
            dxy = sc.tile([P, 2, KMAX * N_OBST * NJ], BF16, name=f"dxy_{t}",
                          tag="dxy")
            x0w = sc.tile([P, KMAX, 2], F32, name=f"x0w_{t}", tag="x0w")
            x0cs = sc.tile([P, KMAX, 2], BF16, name=f"x0cs_{t}", tag="x0cs")

            dxv = dxy[:, 0, 0:K * N_OBST * NJ].rearrange(
                "p (g k j) -> p g k j", k=N_OBST, j=NJ)
            dyv = dxy[:, 1, 0:K * N_OBST * NJ].rearrange(
                "p (g k j) -> p g k j", k=N_OBST, j=NJ)

            # ---- DVE: trig args, u-sums, dx half, vcs, reductions ----
            nc.vector.add_range_wrap(out=ws[:, 0:K, 0, 0:39], in_=th38,
                                     shift=HALF_PI, bound=PI, period=TWO_PI)
            nc.vector.add_range_wrap(out=ws[:, 0:K, 1, 0:39], in_=th38,
                                     shift=0.0, bound=PI, period=TWO_PI)
            nc.vector.reduce_sum(
                out=Q[:, ts, 2:3], in_=uv[:, :, :, 1:2].rearrange(
                    "p g j f -> p g f j"), axis=mybir.AxisListType.X)
            nc.vector.reduce_sum(
                out=Q[:, ts, 3:4], in_=uv[:, :, :, 0:1].rearrange(
                    "p g j f -> p g f j"), axis=mybir.AxisListType.X)
            # dx: first half of groups on DVE, rest on GpSimd
            nc.vector.tensor_sub(out=dxv[:, 0:KH],
                                 in0=_bcast(xv[:, 0:KH, :, 0], 2, N_OBST),
                                 in1=_bcast(ov[:, 0:KH, :, 0], 3, NJ))
            # x0 trig args (from inputs): [cos-arg, sin-arg]
            nc.vector.add_range_wrap(out=x0w[:, 0:K, 0], in_=i_t[:, 0:K, 2],
                                     shift=HALF_PI, bound=PI, period=TWO_PI)
            nc.vector.add_range_wrap(out=x0w[:, 0:K, 1], in_=i_t[:, 0:K, 2],
                                     shift=0.0, bound=PI, period=TWO_PI)

            # ---- ACT: v broadcast, sins, snapshots ----
            nc.scalar.activation(out=vd[:, 0:K, :, 0:39], in_=_bcast(v38, 2, 2),
                                 func=mybir.ActivationFunctionType.Identity)
            nc.scalar.activation(out=cs[:, 0:K, :, 0:39], in_=ws[:, 0:K, :, 0:39],
                                 func=mybir.ActivationFunctionType.Sin)
            nc.scalar.activation(out=x0cs[:, 0:K], in_=x0w[:, 0:K],
                                 func=mybir.ActivationFunctionType.Sin)
            nc.scalar.activation(out=X39[:, ts, :], in_=xv[:, :, 39, :],
                                 func=mybir.ActivationFunctionType.Identity)
            nc.scalar.activation(
                out=RTRS[:, 0:K], in_=ov[:, :, :, 2],
                func=mybir.ActivationFunctionType.Square, bias=CW[:, 0:1],
                accum_out=OUT[:, RPP + NT + 2 + t: RPP + NT + 3 + t])

            # ---- GpSimd: dy, dx second half ----
            nc.gpsimd.tensor_sub(out=dyv,
                                 in0=_bcast(xv[:, :, :, 1], 2, N_OBST),
                                 in1=_bcast(ov[:, :, :, 1], 3, NJ))
            nc.gpsimd.tensor_sub(out=dxv[:, KH:K],
                                 in0=_bcast(xv[:, KH:K, :, 0], 2, N_OBST),
                                 in1=_bcast(ov[:, KH:K, :, 0], 3, NJ))
            # deferred load emissions (buffers now provably free soon)
            if t == 0:
                nc.gpsimd.dma_start(out=t2_t[:, 0:K2],
                                    in_=tgtv[:, OFFS[2]:OFFS[2] + K2])
            if t + 3 <= NT - 1:
                emit_loads(t + 3)
            # BASE for previous tile (needs Q complete there)
            if t >= 1:
                pt = t - 1
                Kp, _, _, i_p = tl[pt][0], None, None, tl[pt][3]
                BASE = sc.tile([P, KMAX, 4], F32, name=f"base_{t-1}", tag="base")
                nc.gpsimd.tensor_sub(out=BASE[:, 0:Kp], in0=X39[:, OFFS[pt]:OFFS[pt] + Kp, :],
                                     in1=i_p[:, 0:Kp, 0:4])
                basel[pt] = BASE

            # ---- DVE: vcs (in-place into cs), q_c/q_s, d2, x0 terms, diff ----
            nc.vector.tensor_mul(out=cs[:, 0:K, :, 0:39], in0=cs[:, 0:K, :, 0:39],
                                 in1=vd[:, 0:K, :, 0:39])
            nc.vector.reduce_sum(out=Q[:, ts, 0:2], in_=cs[:, 0:K, :, 0:39],
                                 axis=mybir.AxisListType.X)
            # x0vcs = v0 * [cos th0; sin th0]; Q[:, ts, 0:2] += x0vcs
            nc.vector.tensor_mul(out=x0cs[:, 0:K], in0=x0cs[:, 0:K],
                                 in1=_bcast(i_t[:, 0:K, 3], 2, 2))
            nc.vector.tensor_add(out=Q[:, ts, 0:2], in0=Q[:, ts, 0:2],
                                 in1=x0cs[:, 0:K])
            # d2 into the staging area (custom: dx^2 + dy^2, one op)
            nc.vector._custom_dve(
                sqsqadd, out=D2[:, o * N_OBST * NJ:(o + K) * N_OBST * NJ],
                in0=dxy[:, 0, 0:K * N_OBST * NJ], in1=dxy[:, 1, 0:K * N_OBST * NJ])
            # diff in-place into p_t (after all x/u readers)
            nc.vector.tensor_sub(out=p_t[:, 0:K], in0=p_t[:, 0:K],
                                 in1=t_t[:, 0:K])
            # mse: Square + accumulate
            nc.scalar.activation(
                out=p_t[:, 0:K], in_=p_t[:, 0:K],
                func=mybir.ActivationFunctionType.Square,
                accum_out=OUT[:, RPP + t: RPP + t + 1])

            if t >= 1:
                phase_b_finish(t - 1)

        # last tile's BASE + phase B
        K4 = KS[NT - 1]
        BASE4 = sc.tile([P, KMAX, 4], F32, name="base_4", tag="base")
        nc.gpsimd.tensor_sub(out=BASE4[:, 0:K4],
                             in0=X39[:, OFFS[NT - 1]:OFFS[NT - 1] + K4, :],
                             in1=tl[NT - 1][3][:, 0:K4, 0:4])
        basel[NT - 1] = BASE4
        phase_b_finish(NT - 1)

        # ---- sqrt phase (one table switch) ----
        SPLIT = (OFFS[3]) * N_OBST * NJ  # tiles 0-2 | 3-4
        nc.scalar.activation(
            out=D2[:, 0:SPLIT], in_=D2[:, 0:SPLIT],
            func=mybir.ActivationFunctionType.Sqrt,
            accum_out=OUT[:, RPP + NT: RPP + NT + 1])
        nc.scalar.activation(
            out=D2[:, SPLIT:], in_=D2[:, SPLIT:],
            func=mybir.ActivationFunctionType.Sqrt,
            accum_out=OUT[:, RPP + NT + 1: RPP + NT + 2])
        nc.scalar.activation(out=OUT[:, 0:RPP], in_=DY2[:],
                             func=mybir.ActivationFunctionType.Sqrt)

        nc.sync.dma_start(out=out[:], in_=OUT[:])

    nc.finalize()
    return nc


_NC_CACHE = None


def _get_nc():
    global _NC_CACHE
    if _NC_CACHE is None:
        _NC_CACHE = build_nc()
    return _NC_CACHE


def combine(outs):
    """Host-side reduction of per-core partials (float64)."""
    dy = 0.0
    sq = 0.0
    ob = 0.0
    rad = 0.0
    for o in outs:
        o = o.astype(np.float64)
        dy += o[:, 0:RPP].sum()
        sq += o[:, RPP:RPP + NT].sum()
        ob += o[:, RPP + NT:RPP + NT + 2].sum()
        rad += o[:, RPP + NT + 2:RPP + NT + 2 + NT].sum()
    mse = sq / (B * 240.0)
    constraint = (DT * dy + ob - NJ * rad) / B
    return np.float32(mse + constraint)


def kernel(predictions, targets, inputs):
    nc = _get_nc()
    preds = np.ascontiguousarray(predictions, dtype=np.float32).reshape(
        N_CORES, BC, 240)
    tgts = np.ascontiguousarray(targets, dtype=np.float32).reshape(
        N_CORES, BC, 240)
    inps = np.ascontiguousarray(inputs, dtype=np.float32).reshape(
        N_CORES, BC, 13)
    in_maps = [
        {"predictions": preds[c], "targets": tgts[c], "inputs": inps[c]}
        for c in range(N_CORES)
    ]
    res = run_bass_kernel_spmd(nc, in_maps, core_ids=list(range(N_CORES)))
    return combine([r["out"] for r in res.results])
